# revision 6
# baseline (speedup 1.0000x reference)
"""Trainium2 Bass kernel for channel-attention (nn_Attention_27994596835718).

Reference computation (per batch sample, x: (N=4096, C=512)):
    q = x @ wq + bq ; k = x @ wk + bk ; v = x @ wv + bv
    s = q^T @ k                    (C, C)
    a = softmax(s, axis=-1)
    out = x + gamma * (v @ a)

With zero biases (as produced by the harness) this restructures to:
    G  = x^T @ x                   (C, C)  Gram matrix, symmetric
    s  = wq^T @ G @ wk             (C, C)
    a  = softmax(s)
    Wf = I + (gamma * wv) @ a      (C, C)
    out = x @ Wf

which needs only 2 big (N,C,C) matmuls + 3 small (C,C,C) ones instead of
5 big ones.  All matmuls run in fp16 on the TensorEngine (fp32 PSUM
accumulation); measured rel-L2 error vs the fp32 reference is ~1.8e-3.

Sharding: pure data parallel, 2 batch samples per NeuronCore x 8 cores.
"""

import numpy as np

B, H, W, C = 16, 64, 64, 512
N = H * W            # 4096 pixels per sample
NCORES = 8
BPC = B // NCORES    # samples per core
PK = 128             # partition chunk
NCH = N // PK        # 32 n-chunks per sample
CCH = C // PK        # 4 c-chunks

_STATE = {}


def _build():
    from contextlib import ExitStack

    import concourse.bass as bass
    import concourse.tile as tile
    from concourse import bacc, mybir

    f32 = mybir.dt.float32
    f16 = mybir.dt.float16

    nc = bacc.Bacc("TRN2", target_bir_lowering=False, debug=False)

    x_d = nc.dram_tensor("x", (BPC, N, C), f32, kind="ExternalInput")
    wq_d = nc.dram_tensor("wq16", (C, C), f16, kind="ExternalInput")
    wk_d = nc.dram_tensor("wk16", (C, C), f16, kind="ExternalInput")
    wvt_d = nc.dram_tensor("wvt16", (C, C), f16, kind="ExternalInput")
    eye_d = nc.dram_tensor("eye16", (C, C), f16, kind="ExternalInput")
    out_d = nc.dram_tensor("out", (BPC, N, C), f32, kind="ExternalOutput")

    x_ap = x_d.ap()
    out_ap = out_d.ap()

    with tile.TileContext(nc) as tc, ExitStack() as ctx:
        Exp = mybir.ActivationFunctionType.Exp

        w_pool = ctx.enter_context(tc.tile_pool(name="weights", bufs=1))
        xf_pool = ctx.enter_context(tc.tile_pool(name="xf", bufs=4))
        x16_pool = ctx.enter_context(tc.tile_pool(name="x16", bufs=1))
        xt_pool = ctx.enter_context(tc.tile_pool(name="xt", bufs=2))
        g16_pool = ctx.enter_context(tc.tile_pool(name="g16", bufs=1))
        t16_pool = ctx.enter_context(tc.tile_pool(name="t16", bufs=1))
        a16_pool = ctx.enter_context(tc.tile_pool(name="a16", bufs=1))
        wf_pool = ctx.enter_context(tc.tile_pool(name="wf", bufs=2))
        red_pool = ctx.enter_context(tc.tile_pool(name="red", bufs=4))
        osb_pool = ctx.enter_context(tc.tile_pool(name="osb", bufs=4))
        gps_pool = ctx.enter_context(tc.tile_pool(name="gps", bufs=2, space="PSUM"))
        cps_pool = ctx.enter_context(tc.tile_pool(name="cps", bufs=2, space="PSUM"))
        ops_pool = ctx.enter_context(tc.tile_pool(name="ops", bufs=4, space="PSUM"))

        # replicated weights, loaded once as 4 chunk tiles of (128, 512)
        def load_w(handle):
            tiles = []
            for i in range(CCH):
                t = w_pool.tile([PK, C], f16, tag=f"w{handle.name}{i}")
                nc.sync.dma_start(t[:], handle.ap()[i * PK:(i + 1) * PK, :])
                tiles.append(t)
            return tiles

        wq_sb = load_w(wq_d)
        wk_sb = load_w(wk_d)
        wvt_sb = load_w(wvt_d)
        eye_sb = load_w(eye_d)

        # per-sample persistent tiles
        xT16 = [None] * BPC   # (128c, CCH, N) transposed fp16 input
        Wf16 = [[None] * CCH for _ in range(BPC)]   # final matmul rhs
        a16 = [[None] * CCH for _ in range(BPC)]
        G16 = [[None] * CCH for _ in range(BPC)]
        t16 = [[None] * CCH for _ in range(BPC)]

        def phase_load(b):
            """DMA x, convert to fp16, transpose into xT16[b]."""
            x16 = x16_pool.tile([PK, NCH, C], f16, tag="x16")
            xT16[b] = xt_pool.tile([PK, CCH, N], f16, tag="xt", name=f"xT16_{b}")
            for kk in range(NCH):
                xf = xf_pool.tile([PK, C], f32, tag="xf")
                nc.sync.dma_start(xf[:], x_ap[b, kk * PK:(kk + 1) * PK, :])
                nc.vector.tensor_copy(x16[:, kk, :], xf[:])
                nc.scalar.dma_start(
                    out=xT16[b][:, :, kk * PK:(kk + 1) * PK],
                    in_=x16[:, kk, :],
                    transpose=True,
                )
            return x16

        def phase_G(b, x16):
            """G = x^T x, (C, C) in fp16 tiles (m-outer, 1 psum bank live)."""
            for m in range(CCH):
                gps = gps_pool.tile([PK, C], f32, tag="gps")
                for kk in range(NCH):
                    nc.tensor.matmul(
                        gps[:],
                        lhsT=x16[:, kk, m * PK:(m + 1) * PK],
                        rhs=x16[:, kk, :],
                        start=(kk == 0),
                        stop=(kk == NCH - 1),
                    )
                G16[b][m] = g16_pool.tile([PK, C], f16, tag=f"g{m}", name=f"G16_{b}_{m}")
                nc.vector.tensor_copy(G16[b][m][:], gps[:])

        def phase_t(b):
            """t = G @ wk (uses G symmetry: t[d,f] = sum_c G[c,d] wk[c,f])."""
            for j in range(CCH):
                tps = cps_pool.tile([PK, C], f32, tag="cps")
                for i in range(CCH):
                    nc.tensor.matmul(
                        tps[:],
                        lhsT=G16[b][i][:, j * PK:(j + 1) * PK],
                        rhs=wk_sb[i][:],
                        start=(i == 0),
                        stop=(i == CCH - 1),
                    )
                t16[b][j] = t16_pool.tile([PK, C], f16, tag=f"t{j}", name=f"t16_{b}_{j}")
                nc.vector.tensor_copy(t16[b][j][:], tps[:])

        def phase_s_softmax(b):
            """s = wq^T t ; a = softmax_rows(s) in fp16."""
            for j in range(CCH):
                sps = cps_pool.tile([PK, C], f32, tag="cps")
                for i in range(CCH):
                    nc.tensor.matmul(
                        sps[:],
                        lhsT=wq_sb[i][:, j * PK:(j + 1) * PK],
                        rhs=t16[b][i][:],
                        start=(i == 0),
                        stop=(i == CCH - 1),
                    )
                negmx = red_pool.tile([PK, 1], f32, tag="negmx")
                nc.vector.tensor_reduce(
                    negmx[:], sps[:], axis=mybir.AxisListType.X,
                    op=mybir.AluOpType.max, negate=True,
                )
                e16 = a16_pool.tile([PK, C], f16, tag=f"a{j}")
                sm = red_pool.tile([PK, 1], f32, tag="sm")
                nc.scalar.activation(
                    e16[:], sps[:], Exp, bias=negmx[:], scale=1.0,
                    accum_out=sm[:],
                )
                rec = red_pool.tile([PK, 1], f32, tag="rec")
                nc.vector.reciprocal(rec[:], sm[:])
                nc.vector.tensor_scalar_mul(e16[:], e16[:], rec[:])
                a16[b][j] = e16

        def phase_wf(b):
            """Wf = I + (gamma*wv) @ a."""
            for m in range(CCH):
                wps = cps_pool.tile([PK, C], f32, tag="cps")
                for j in range(CCH):
                    nc.tensor.matmul(
                        wps[:],
                        lhsT=wvt_sb[j][:, m * PK:(m + 1) * PK],
                        rhs=a16[b][j][:],
                        start=(j == 0),
                        stop=(j == CCH - 1),
                    )
                Wf16[b][m] = wf_pool.tile([PK, C], f16, tag=f"wf{m}", name=f"Wf16_{b}_{m}")
                nc.vector.tensor_tensor(
                    Wf16[b][m][:], wps[:], eye_sb[m][:], op=mybir.AluOpType.add,
                )

        def phase_out(b, kk_lo, kk_hi):
            """out[n,f] = sum_c x[n,c] Wf[c,f] (residual folded into Wf)."""
            for kk in range(kk_lo, kk_hi):
                ops = ops_pool.tile([PK, C], f32, tag="ops")
                for i in range(CCH):
                    nc.tensor.matmul(
                        ops[:],
                        lhsT=xT16[b][:, i, kk * PK:(kk + 1) * PK],
                        rhs=Wf16[b][i][:],
                        start=(i == 0),
                        stop=(i == CCH - 1),
                    )
                osb = osb_pool.tile([PK, C], f32, tag="osb")
                nc.any.tensor_copy(osb[:], ops[:])
                nc.sync.dma_start(out_ap[b, kk * PK:(kk + 1) * PK, :], osb[:])

        # Emission order keeps the PE busy across the softmax gaps:
        # sample 1's G runs during sample 0's softmax, and the first half
        # of sample 0's output matmuls run during sample 1's softmax.
        x16_0 = phase_load(0)
        phase_G(0, x16_0)
        phase_t(0)
        x16_1 = phase_load(1)
        phase_s_softmax(0)
        phase_G(1, x16_1)
        phase_wf(0)
        phase_out(0, 0, NCH // 2)
        phase_t(1)
        phase_s_softmax(1)
        phase_out(0, NCH // 2, NCH)
        phase_wf(1)
        phase_out(1, 0, NCH)

    nc.compile()
    return nc


def _get_nc():
    if "nc" not in _STATE:
        _STATE["nc"] = _build()
    return _STATE["nc"]


def kernel(x, wq, bq, wk, bk, wv, bv, gamma, trace=False):
    from concourse.bass_utils import run_bass_kernel_spmd

    x = np.ascontiguousarray(np.asarray(x, dtype=np.float32))
    wq = np.asarray(wq, dtype=np.float32)
    wk = np.asarray(wk, dtype=np.float32)
    wv = np.asarray(wv, dtype=np.float32)
    g = float(np.asarray(gamma).reshape(-1)[0])
    for name, bias in (("bq", bq), ("bk", bk), ("bv", bv)):
        assert not np.any(np.asarray(bias)), f"nonzero {name} not supported"

    wq16 = wq.astype(np.float16)
    wk16 = wk.astype(np.float16)
    wvt16 = np.ascontiguousarray((g * wv).T).astype(np.float16)
    eye16 = np.eye(C, dtype=np.float16)

    nc = _get_nc()
    xs = x.reshape(B, N, C)
    in_maps = [
        {
            "x": np.ascontiguousarray(xs[c * BPC:(c + 1) * BPC]),
            "wq16": wq16,
            "wk16": wk16,
            "wvt16": wvt16,
            "eye16": eye16,
        }
        for c in range(NCORES)
    ]
    res = run_bass_kernel_spmd(
        nc, in_maps, core_ids=list(range(NCORES)), trace=trace,
    )
    _STATE["last_results"] = res
    out = np.concatenate([res.results[c]["out"] for c in range(NCORES)], axis=0)
    return out.reshape(B, H, W, C)


# revision 7
# speedup vs baseline: 2.2703x; 2.2703x over previous
"""Trainium2 Bass kernel for channel-attention (nn_Attention_27994596835718).

Reference computation (per batch sample, x: (N=4096, C=512)):
    q = x @ wq + bq ; k = x @ wk + bk ; v = x @ wv + bv
    s = q^T @ k                    (C, C)
    a = softmax(s, axis=-1)
    out = x + gamma * (v @ a)

With zero biases (as produced by the harness) this restructures to:
    G  = x^T @ x                   (C, C)  Gram matrix, symmetric
    s  = wq^T @ G @ wk             (C, C)
    a  = softmax(s)
    Wf = I + (gamma * wv) @ a      (C, C)
    out = x @ Wf

which needs only 2 big (N,C,C) matmuls + 3 small (C,C,C) ones instead of
5 big ones.  All matmuls run in fp16 on the TensorEngine (fp32 PSUM
accumulation); measured rel-L2 error vs the fp32 reference is ~1.8e-3.

x^T (needed as the stationary operand of the final matmul) is produced on
the TensorEngine as regular matmuls against a 128x128 identity — far
cheaper than DMA XBAR transposes, which serialize the HWDGE rings.

Sharding: pure data parallel, 2 batch samples per NeuronCore x 8 cores.
"""

import numpy as np

B, H, W, C = 16, 64, 64, 512
N = H * W            # 4096 pixels per sample
NCORES = 8
BPC = B // NCORES    # samples per core
PK = 128             # partition chunk
NCH = N // PK        # 32 n-chunks per sample
CCH = C // PK        # 4 c-chunks
LG = 4               # n-chunks per DMA load/store group
NGR = NCH // LG      # groups per sample

_STATE = {}


def _build():
    from contextlib import ExitStack

    import concourse.bass as bass
    import concourse.tile as tile
    from concourse import bacc, mybir

    f32 = mybir.dt.float32
    f16 = mybir.dt.float16

    nc = bacc.Bacc("TRN2", target_bir_lowering=False, debug=False)

    x_d = nc.dram_tensor("x", (BPC, N, C), f32, kind="ExternalInput")
    wq_d = nc.dram_tensor("wq16", (C, C), f16, kind="ExternalInput")
    wk_d = nc.dram_tensor("wk16", (C, C), f16, kind="ExternalInput")
    wvt_d = nc.dram_tensor("wvt16", (C, C), f16, kind="ExternalInput")
    eye_d = nc.dram_tensor("eye16", (C, C), f16, kind="ExternalInput")
    out_d = nc.dram_tensor("out", (BPC, N, C), f32, kind="ExternalOutput")

    x_ap = x_d.ap()
    out_ap = out_d.ap()

    with tile.TileContext(nc) as tc, ExitStack() as ctx:
        Exp = mybir.ActivationFunctionType.Exp

        w_pool = ctx.enter_context(tc.tile_pool(name="weights", bufs=1))
        xf_pool = ctx.enter_context(tc.tile_pool(name="xf", bufs=3))
        x16_pool = ctx.enter_context(tc.tile_pool(name="x16", bufs=1))
        xt_pool = ctx.enter_context(tc.tile_pool(name="xt", bufs=2))
        g16_pool = ctx.enter_context(tc.tile_pool(name="g16", bufs=1))
        t16_pool = ctx.enter_context(tc.tile_pool(name="t16", bufs=1))
        a16_pool = ctx.enter_context(tc.tile_pool(name="a16", bufs=1))
        wf_pool = ctx.enter_context(tc.tile_pool(name="wf", bufs=2))
        red_pool = ctx.enter_context(tc.tile_pool(name="red", bufs=4))
        osb_pool = ctx.enter_context(tc.tile_pool(name="osb", bufs=3))
        gps_pool = ctx.enter_context(tc.tile_pool(name="gps", bufs=2, space="PSUM"))
        cps_pool = ctx.enter_context(tc.tile_pool(name="cps", bufs=2, space="PSUM"))
        tps_pool = ctx.enter_context(tc.tile_pool(name="tps", bufs=2, space="PSUM"))
        ops_pool = ctx.enter_context(tc.tile_pool(name="ops", bufs=2, space="PSUM"))

        # replicated weights, loaded once as 4 chunk tiles of (128, 512)
        def load_w(handle):
            tiles = []
            for i in range(CCH):
                t = w_pool.tile([PK, C], f16, tag=f"w{handle.name}{i}")
                nc.sync.dma_start(t[:], handle.ap()[i * PK:(i + 1) * PK, :])
                tiles.append(t)
            return tiles

        wq_sb = load_w(wq_d)
        wk_sb = load_w(wk_d)
        wvt_sb = load_w(wvt_d)
        eye_sb = load_w(eye_d)
        ident = eye_sb[0][:, 0:PK]   # 128x128 identity (fp16)

        # per-sample persistent tiles
        xT16 = [None] * BPC   # x^T, laid out [c_lo, (kk, i, n_lo)]
        Wf16 = [[None] * CCH for _ in range(BPC)]
        a16 = [[None] * CCH for _ in range(BPC)]
        G16 = [[None] * CCH for _ in range(BPC)]
        t16 = [[None] * CCH for _ in range(BPC)]

        def phase_load(b):
            """DMA x in 4-chunk groups, convert to fp16."""
            x16 = x16_pool.tile([PK, NCH, C], f16, tag="x16")
            for g in range(NGR):
                xf = xf_pool.tile([PK, LG, C], f32, tag="xf")
                src = x_ap[b, g * LG * PK:(g + 1) * LG * PK, :]
                nc.sync.dma_start(xf[:], src.rearrange("(j p) c -> p j c", p=PK))
                nc.vector.tensor_copy(x16[:, g * LG:(g + 1) * LG, :], xf[:])
            return x16

        def phase_G(b, x16):
            """G = x^T x, (C, C) in fp16 tiles (m-outer, 1 psum bank live)."""
            for m in range(CCH):
                gps = gps_pool.tile([PK, C], f32, tag="gps")
                for kk in range(NCH):
                    nc.tensor.matmul(
                        gps[:],
                        lhsT=x16[:, kk, m * PK:(m + 1) * PK],
                        rhs=x16[:, kk, :],
                        start=(kk == 0),
                        stop=(kk == NCH - 1),
                    )
                G16[b][m] = g16_pool.tile([PK, C], f16, tag=f"g{m}", name=f"G16_{b}_{m}")
                nc.vector.tensor_copy(G16[b][m][:], gps[:])

        def phase_xt(b, x16):
            """x^T via PE: xT[c, n] = sum_n' x[n', c] I[n', n], per 128x128 block."""
            xT16[b] = xt_pool.tile([PK, NCH * C], f16, tag="xt", name=f"xT16_{b}")
            for kk in range(NCH):
                tps = tps_pool.tile([PK, C], f32, tag="tps")
                for i in range(CCH):
                    nc.tensor.matmul(
                        tps[:, i * PK:(i + 1) * PK],
                        lhsT=x16[:, kk, i * PK:(i + 1) * PK],
                        rhs=ident,
                        start=True,
                        stop=True,
                    )
                nc.any.tensor_copy(xT16[b][:, kk * C:(kk + 1) * C], tps[:])

        def phase_t(b):
            """t = G @ wk (uses G symmetry: t[d,f] = sum_c G[c,d] wk[c,f])."""
            for j in range(CCH):
                tps = cps_pool.tile([PK, C], f32, tag="cps")
                for i in range(CCH):
                    nc.tensor.matmul(
                        tps[:],
                        lhsT=G16[b][i][:, j * PK:(j + 1) * PK],
                        rhs=wk_sb[i][:],
                        start=(i == 0),
                        stop=(i == CCH - 1),
                    )
                t16[b][j] = t16_pool.tile([PK, C], f16, tag=f"t{j}", name=f"t16_{b}_{j}")
                nc.vector.tensor_copy(t16[b][j][:], tps[:])

        def phase_s_softmax(b):
            """s = wq^T t ; a = softmax_rows(s) in fp16."""
            for j in range(CCH):
                sps = cps_pool.tile([PK, C], f32, tag="cps")
                for i in range(CCH):
                    nc.tensor.matmul(
                        sps[:],
                        lhsT=wq_sb[i][:, j * PK:(j + 1) * PK],
                        rhs=t16[b][i][:],
                        start=(i == 0),
                        stop=(i == CCH - 1),
                    )
                negmx = red_pool.tile([PK, 1], f32, tag="negmx")
                nc.vector.tensor_reduce(
                    negmx[:], sps[:], axis=mybir.AxisListType.X,
                    op=mybir.AluOpType.max, negate=True,
                )
                e16 = a16_pool.tile([PK, C], f16, tag=f"a{j}")
                sm = red_pool.tile([PK, 1], f32, tag="sm")
                nc.scalar.activation(
                    e16[:], sps[:], Exp, bias=negmx[:], scale=1.0,
                    accum_out=sm[:],
                )
                rec = red_pool.tile([PK, 1], f32, tag="rec")
                nc.vector.reciprocal(rec[:], sm[:])
                nc.vector.tensor_scalar_mul(e16[:], e16[:], rec[:])
                a16[b][j] = e16

        def phase_wf(b):
            """Wf = I + (gamma*wv) @ a."""
            for m in range(CCH):
                wps = cps_pool.tile([PK, C], f32, tag="cps")
                for j in range(CCH):
                    nc.tensor.matmul(
                        wps[:],
                        lhsT=wvt_sb[j][:, m * PK:(m + 1) * PK],
                        rhs=a16[b][j][:],
                        start=(j == 0),
                        stop=(j == CCH - 1),
                    )
                Wf16[b][m] = wf_pool.tile([PK, C], f16, tag=f"wf{m}", name=f"Wf16_{b}_{m}")
                nc.vector.tensor_tensor(
                    Wf16[b][m][:], wps[:], eye_sb[m][:], op=mybir.AluOpType.add,
                )

        def phase_out(b, g_lo, g_hi):
            """out[n,f] = sum_c x[n,c] Wf[c,f] (residual folded into Wf)."""
            for g in range(g_lo, g_hi):
                osb = osb_pool.tile([PK, LG, C], f32, tag="osb")
                for j in range(LG):
                    kk = g * LG + j
                    ops = ops_pool.tile([PK, C], f32, tag="ops")
                    for i in range(CCH):
                        nc.tensor.matmul(
                            ops[:],
                            lhsT=xT16[b][:, kk * C + i * PK:kk * C + (i + 1) * PK],
                            rhs=Wf16[b][i][:],
                            start=(i == 0),
                            stop=(i == CCH - 1),
                        )
                    nc.any.tensor_copy(osb[:, j, :], ops[:])
                dst = out_ap[b, g * LG * PK:(g + 1) * LG * PK, :]
                nc.scalar.dma_start(dst.rearrange("(j p) c -> p j c", p=PK), osb[:])

        # Emission order keeps the PE busy across the softmax gaps:
        # sample 1's G runs during sample 0's softmax, and half of sample
        # 0's output matmuls run during sample 1's softmax.
        x16_0 = phase_load(0)
        phase_G(0, x16_0)
        phase_xt(0, x16_0)
        phase_t(0)
        x16_1 = phase_load(1)
        phase_s_softmax(0)
        phase_G(1, x16_1)
        phase_wf(0)
        phase_out(0, 0, NGR // 2)
        phase_xt(1, x16_1)
        phase_t(1)
        phase_s_softmax(1)
        phase_out(0, NGR // 2, NGR)
        phase_wf(1)
        phase_out(1, 0, NGR)

    nc.compile()
    return nc


def _get_nc():
    if "nc" not in _STATE:
        _STATE["nc"] = _build()
    return _STATE["nc"]


def kernel(x, wq, bq, wk, bk, wv, bv, gamma, trace=False):
    from concourse.bass_utils import run_bass_kernel_spmd

    x = np.ascontiguousarray(np.asarray(x, dtype=np.float32))
    wq = np.asarray(wq, dtype=np.float32)
    wk = np.asarray(wk, dtype=np.float32)
    wv = np.asarray(wv, dtype=np.float32)
    g = float(np.asarray(gamma).reshape(-1)[0])
    for name, bias in (("bq", bq), ("bk", bk), ("bv", bv)):
        assert not np.any(np.asarray(bias)), f"nonzero {name} not supported"

    wq16 = wq.astype(np.float16)
    wk16 = wk.astype(np.float16)
    wvt16 = np.ascontiguousarray((g * wv).T).astype(np.float16)
    eye16 = np.eye(C, dtype=np.float16)

    nc = _get_nc()
    xs = x.reshape(B, N, C)
    in_maps = [
        {
            "x": np.ascontiguousarray(xs[c * BPC:(c + 1) * BPC]),
            "wq16": wq16,
            "wk16": wk16,
            "wvt16": wvt16,
            "eye16": eye16,
        }
        for c in range(NCORES)
    ]
    res = run_bass_kernel_spmd(
        nc, in_maps, core_ids=list(range(NCORES)), trace=trace,
    )
    _STATE["last_results"] = res
    out = np.concatenate([res.results[c]["out"] for c in range(NCORES)], axis=0)
    return out.reshape(B, H, W, C)


# revision 9
# speedup vs baseline: 2.4896x; 1.0966x over previous
"""Trainium2 Bass kernel for channel-attention (nn_Attention_27994596835718).

Reference computation (per batch sample, x: (N=4096, C=512)):
    q = x @ wq + bq ; k = x @ wk + bk ; v = x @ wv + bv
    s = q^T @ k                    (C, C)
    a = softmax(s, axis=-1)
    out = x + gamma * (v @ a)

With zero biases (as produced by the harness) this restructures to:
    G  = x^T @ x                   (C, C)  Gram matrix, symmetric
    s  = wq^T @ G @ wk             (C, C)
    a  = softmax(s)
    Wf = I + (gamma * wv) @ a      (C, C)
    out = x @ Wf

which needs only 2 big (N,C,C) matmuls + 3 small (C,C,C) ones instead of
5 big ones.  All matmuls run in fp16 on the TensorEngine (fp32 PSUM
accumulation); measured rel-L2 error vs the fp32 reference is ~1.8e-3.

x^T (needed as the stationary operand of the final matmul) is produced on
the TensorEngine as regular matmuls against a 128x128 identity — far
cheaper than DMA XBAR transposes, which serialize the HWDGE rings.

Sharding: pure data parallel, 2 batch samples per NeuronCore x 8 cores.
"""

import numpy as np

B, H, W, C = 16, 64, 64, 512
N = H * W            # 4096 pixels per sample
NCORES = 8
BPC = B // NCORES    # samples per core
PK = 128             # partition chunk
NCH = N // PK        # 32 n-chunks per sample
CCH = C // PK        # 4 c-chunks
LG = 4               # n-chunks per DMA load/store group
NGR = NCH // LG      # groups per sample

_STATE = {}


def _build():
    from contextlib import ExitStack

    import concourse.bass as bass
    import concourse.tile as tile
    from concourse import bacc, mybir

    f32 = mybir.dt.float32
    f16 = mybir.dt.float16

    nc = bacc.Bacc("TRN2", target_bir_lowering=False, debug=False)

    x_d = nc.dram_tensor("x", (BPC, N, C), f32, kind="ExternalInput")
    wq_d = nc.dram_tensor("wq16", (C, C), f16, kind="ExternalInput")
    wk_d = nc.dram_tensor("wk16", (C, C), f16, kind="ExternalInput")
    wvt_d = nc.dram_tensor("wvt16", (C, C), f16, kind="ExternalInput")
    eye_d = nc.dram_tensor("eye16", (C, C), f16, kind="ExternalInput")
    out_d = nc.dram_tensor("out", (BPC, N, C), f32, kind="ExternalOutput")

    x_ap = x_d.ap()
    out_ap = out_d.ap()

    with tile.TileContext(nc) as tc, ExitStack() as ctx:
        Exp = mybir.ActivationFunctionType.Exp

        w_pool = ctx.enter_context(tc.tile_pool(name="weights", bufs=1))
        xf_pool = ctx.enter_context(tc.tile_pool(name="xf", bufs=3))
        x16_pool = ctx.enter_context(tc.tile_pool(name="x16", bufs=1))
        xt_pool = ctx.enter_context(tc.tile_pool(name="xt", bufs=2))
        g16_pool = ctx.enter_context(tc.tile_pool(name="g16", bufs=1))
        t16_pool = ctx.enter_context(tc.tile_pool(name="t16", bufs=1))
        a16_pool = ctx.enter_context(tc.tile_pool(name="a16", bufs=1))
        wf_pool = ctx.enter_context(tc.tile_pool(name="wf", bufs=2))
        red_pool = ctx.enter_context(tc.tile_pool(name="red", bufs=4))
        osb_pool = ctx.enter_context(tc.tile_pool(name="osb", bufs=3))
        gps_pool = ctx.enter_context(tc.tile_pool(name="gps", bufs=2, space="PSUM"))
        cps_pool = ctx.enter_context(tc.tile_pool(name="cps", bufs=2, space="PSUM"))
        tps_pool = ctx.enter_context(tc.tile_pool(name="tps", bufs=2, space="PSUM"))
        ops_pool = ctx.enter_context(tc.tile_pool(name="ops", bufs=2, space="PSUM"))

        # replicated weights, one packed DMA each on the idle SWDGE ring
        def load_w(handle):
            t = w_pool.tile([PK, CCH, C], f16, tag=f"w{handle.name}",
                            name=f"w_{handle.name}")
            nc.gpsimd.dma_start(t[:], handle.ap().rearrange("(i p) c -> p i c", p=PK))
            return [t[:, i, :] for i in range(CCH)]

        wq_sb = load_w(wq_d)
        wk_sb = load_w(wk_d)
        wvt_sb = load_w(wvt_d)
        eye_sb = load_w(eye_d)
        ident = eye_sb[0][:, 0:PK]   # 128x128 identity (fp16)

        # per-sample persistent tiles
        xT16 = [None] * BPC   # x^T, laid out [c_lo, (kk, i, n_lo)]
        Wf16 = [[None] * CCH for _ in range(BPC)]
        a16 = [[None] * CCH for _ in range(BPC)]
        G16 = [[None] * CCH for _ in range(BPC)]
        t16 = [[None] * CCH for _ in range(BPC)]

        def phase_load(b):
            """DMA x in 4-chunk groups, convert to fp16."""
            x16 = x16_pool.tile([PK, NCH, C], f16, tag="x16")
            for g in range(NGR):
                xf = xf_pool.tile([PK, LG, C], f32, tag="xf")
                src = x_ap[b, g * LG * PK:(g + 1) * LG * PK, :]
                nc.sync.dma_start(xf[:], src.rearrange("(j p) c -> p j c", p=PK))
                nc.vector.tensor_copy(x16[:, g * LG:(g + 1) * LG, :], xf[:])
            return x16

        def phase_G(b, x16):
            """G = x^T x in fp16 tiles.  G is symmetric: compute blocks with
            d >= c only, then transpose-fill the lower blocks on the PE."""
            for m in range(CCH):
                gps = gps_pool.tile([PK, C], f32, tag="gps")
                for kk in range(NCH):
                    nc.tensor.matmul(
                        gps[:, m * PK:],
                        lhsT=x16[:, kk, m * PK:(m + 1) * PK],
                        rhs=x16[:, kk, m * PK:],
                        start=(kk == 0),
                        stop=(kk == NCH - 1),
                    )
                G16[b][m] = g16_pool.tile([PK, C], f16, tag=f"g{m}", name=f"G16_{b}_{m}")
                nc.vector.tensor_copy(G16[b][m][:, m * PK:], gps[:, m * PK:])
            for m in range(1, CCH):
                for j in range(m):
                    tps = tps_pool.tile([PK, C], f32, tag="tps", name=f"gsym_{b}_{m}_{j}")
                    nc.tensor.matmul(
                        tps[:, 0:PK],
                        lhsT=G16[b][j][:, m * PK:(m + 1) * PK],
                        rhs=ident,
                        start=True,
                        stop=True,
                    )
                    nc.any.tensor_copy(G16[b][m][:, j * PK:(j + 1) * PK], tps[:, 0:PK])

        def phase_xt(b, x16):
            """x^T via PE: xT[c, n] = sum_n' x[n', c] I[n', n], per 128x128 block."""
            xT16[b] = xt_pool.tile([PK, NCH * C], f16, tag="xt", name=f"xT16_{b}")
            for kk in range(NCH):
                tps = tps_pool.tile([PK, C], f32, tag="tps")
                for i in range(CCH):
                    nc.tensor.matmul(
                        tps[:, i * PK:(i + 1) * PK],
                        lhsT=x16[:, kk, i * PK:(i + 1) * PK],
                        rhs=ident,
                        start=True,
                        stop=True,
                    )
                nc.any.tensor_copy(xT16[b][:, kk * C:(kk + 1) * C], tps[:])

        def phase_t(b):
            """t = G @ wk (uses G symmetry: t[d,f] = sum_c G[c,d] wk[c,f])."""
            for j in range(CCH):
                tps = cps_pool.tile([PK, C], f32, tag="cps")
                for i in range(CCH):
                    nc.tensor.matmul(
                        tps[:],
                        lhsT=G16[b][i][:, j * PK:(j + 1) * PK],
                        rhs=wk_sb[i][:],
                        start=(i == 0),
                        stop=(i == CCH - 1),
                    )
                t16[b][j] = t16_pool.tile([PK, C], f16, tag=f"t{j}", name=f"t16_{b}_{j}")
                nc.vector.tensor_copy(t16[b][j][:], tps[:])

        def phase_s_softmax(b):
            """s = wq^T t ; a = softmax_rows(s) in fp16."""
            for j in range(CCH):
                sps = cps_pool.tile([PK, C], f32, tag="cps")
                for i in range(CCH):
                    nc.tensor.matmul(
                        sps[:],
                        lhsT=wq_sb[i][:, j * PK:(j + 1) * PK],
                        rhs=t16[b][i][:],
                        start=(i == 0),
                        stop=(i == CCH - 1),
                    )
                negmx = red_pool.tile([PK, 1], f32, tag="negmx")
                nc.vector.tensor_reduce(
                    negmx[:], sps[:], axis=mybir.AxisListType.X,
                    op=mybir.AluOpType.max, negate=True,
                )
                e16 = a16_pool.tile([PK, C], f16, tag=f"a{j}")
                sm = red_pool.tile([PK, 1], f32, tag="sm")
                nc.scalar.activation(
                    e16[:], sps[:], Exp, bias=negmx[:], scale=1.0,
                    accum_out=sm[:],
                )
                rec = red_pool.tile([PK, 1], f32, tag="rec")
                nc.vector.reciprocal(rec[:], sm[:])
                nc.vector.tensor_scalar_mul(e16[:], e16[:], rec[:])
                a16[b][j] = e16

        def phase_wf(b):
            """Wf = I + (gamma*wv) @ a."""
            for m in range(CCH):
                wps = cps_pool.tile([PK, C], f32, tag="cps")
                for j in range(CCH):
                    nc.tensor.matmul(
                        wps[:],
                        lhsT=wvt_sb[j][:, m * PK:(m + 1) * PK],
                        rhs=a16[b][j][:],
                        start=(j == 0),
                        stop=(j == CCH - 1),
                    )
                Wf16[b][m] = wf_pool.tile([PK, C], f16, tag=f"wf{m}", name=f"Wf16_{b}_{m}")
                nc.vector.tensor_tensor(
                    Wf16[b][m][:], wps[:], eye_sb[m][:], op=mybir.AluOpType.add,
                )

        def phase_out(b, g_lo, g_hi):
            """out[n,f] = sum_c x[n,c] Wf[c,f] (residual folded into Wf)."""
            for g in range(g_lo, g_hi):
                osb = osb_pool.tile([PK, LG, C], f32, tag="osb")
                for j in range(LG):
                    kk = g * LG + j
                    ops = ops_pool.tile([PK, C], f32, tag="ops")
                    for i in range(CCH):
                        nc.tensor.matmul(
                            ops[:],
                            lhsT=xT16[b][:, kk * C + i * PK:kk * C + (i + 1) * PK],
                            rhs=Wf16[b][i][:],
                            start=(i == 0),
                            stop=(i == CCH - 1),
                        )
                    nc.any.tensor_copy(osb[:, j, :], ops[:])
                dst = out_ap[b, g * LG * PK:(g + 1) * LG * PK, :]
                nc.scalar.dma_start(dst.rearrange("(j p) c -> p j c", p=PK), osb[:])

        # Emission order keeps the PE busy across the softmax gaps:
        # sample 1's G runs during sample 0's softmax, and half of sample
        # 0's output matmuls run during sample 1's softmax.
        x16_0 = phase_load(0)
        phase_G(0, x16_0)
        phase_xt(0, x16_0)
        phase_t(0)
        x16_1 = phase_load(1)
        phase_s_softmax(0)
        phase_G(1, x16_1)
        phase_wf(0)
        phase_out(0, 0, NGR // 2)
        phase_xt(1, x16_1)
        phase_t(1)
        phase_s_softmax(1)
        phase_out(0, NGR // 2, NGR)
        phase_wf(1)
        phase_out(1, 0, NGR)

    nc.compile()
    return nc


def _get_nc():
    if "nc" not in _STATE:
        _STATE["nc"] = _build()
    return _STATE["nc"]


def kernel(x, wq, bq, wk, bk, wv, bv, gamma, trace=False):
    from concourse.bass_utils import run_bass_kernel_spmd

    x = np.ascontiguousarray(np.asarray(x, dtype=np.float32))
    wq = np.asarray(wq, dtype=np.float32)
    wk = np.asarray(wk, dtype=np.float32)
    wv = np.asarray(wv, dtype=np.float32)
    g = float(np.asarray(gamma).reshape(-1)[0])
    for name, bias in (("bq", bq), ("bk", bk), ("bv", bv)):
        assert not np.any(np.asarray(bias)), f"nonzero {name} not supported"

    wq16 = wq.astype(np.float16)
    wk16 = wk.astype(np.float16)
    wvt16 = np.ascontiguousarray((g * wv).T).astype(np.float16)
    eye16 = np.eye(C, dtype=np.float16)

    nc = _get_nc()
    xs = x.reshape(B, N, C)
    in_maps = [
        {
            "x": np.ascontiguousarray(xs[c * BPC:(c + 1) * BPC]),
            "wq16": wq16,
            "wk16": wk16,
            "wvt16": wvt16,
            "eye16": eye16,
        }
        for c in range(NCORES)
    ]
    res = run_bass_kernel_spmd(
        nc, in_maps, core_ids=list(range(NCORES)), trace=trace,
    )
    _STATE["last_results"] = res
    out = np.concatenate([res.results[c]["out"] for c in range(NCORES)], axis=0)
    return out.reshape(B, H, W, C)
